# revision 31
# baseline (speedup 1.0000x reference)
"""BiDirectionalCrossAttention TRN2 kernel.

Strategy: 8 NeuronCores = 2 attention directions x 4 batch elements, one
(direction, batch) pair per core, zero collectives. Each core runs the full
cross-attention + fusion MLP for its slice.

Layout: activations kept feature-major ("transposed", feature on SBUF
partitions) so every matmul contracts along partitions. Scores computed in
both orientations: [q,k] (softmax reduction + attn-mean output, phase A) and
[k,q] (feeds the aw@V matmul, phase B). Matmuls use fp32r (fp32 with 11-bit
mantissa, full-rate on the PE) except aw@V which uses bf16 (its error is
diluted by the residual path). The attention mask is all-ones per the input
spec and is not applied. Tile pools are opened/closed in strict LIFO order.
"""
import os
import sys

import numpy as np

for _p in ("/opt/trn_rl_repo", "/root/.axon_site/_ro/trn_rl_repo"):
    if os.path.isdir(_p) and _p not in sys.path:
        sys.path.insert(0, _p)

import concourse.bass as bass
import concourse.mybir as mybir
import concourse.tile as tile
from concourse import bacc
from concourse.bass_utils import run_bass_kernel_spmd
from concourse.masks import make_identity
from contextlib import ExitStack

P = 128
L = 1024
D = 1024
H = 16
HD = 64
NS = D // P            # 8 subtiles along a 1024 feature/token dim
CH = 512               # matmul moving-dim chunk (one fp32 PSUM bank)
NCH = L // CH          # 2
F32 = mybir.dt.float32
F32R = mybir.dt.float32r
BF16 = mybir.dt.bfloat16
AF = mybir.ActivationFunctionType
ALU = mybir.AluOpType


def _col_view(ap):
    """[D] dram vector -> [P, NS] (per-partition columns)."""
    return ap.rearrange("(s p) -> p s", p=P)


def _row_tiles(ap):
    """[T, D] dram -> [P, T//P, D] (token-subtile rows)."""
    return ap.rearrange("(ts p) d -> p ts d", p=P)


def build_program():
    nc = bacc.Bacc("TRN2", target_bir_lowering=False, debug=False)

    xq = nc.declare_dram_parameter("xq", [L, D], F32, isOutput=False)
    xkv = nc.declare_dram_parameter("xkv", [L, D], F32, isOutput=False)
    qw = nc.declare_dram_parameter("qw", [D, D], F32, isOutput=False)
    kw = nc.declare_dram_parameter("kw", [D, D], F32, isOutput=False)
    vw = nc.declare_dram_parameter("vw", [D, D], F32, isOutput=False)
    ow = nc.declare_dram_parameter("ow", [D, D], F32, isOutput=False)
    qb = nc.declare_dram_parameter("qb", [D], F32, isOutput=False)
    kb = nc.declare_dram_parameter("kb", [D], F32, isOutput=False)
    vb = nc.declare_dram_parameter("vb", [D], F32, isOutput=False)
    ob = nc.declare_dram_parameter("ob", [D], F32, isOutput=False)
    gg = nc.declare_dram_parameter("gg", [D], F32, isOutput=False)
    gbeta = nc.declare_dram_parameter("gbeta", [D], F32, isOutput=False)
    temp = nc.declare_dram_parameter("temp", [1], F32, isOutput=False)
    w1 = nc.declare_dram_parameter("w1", [2 * D, D], F32, isOutput=False)
    b1 = nc.declare_dram_parameter("b1", [D], F32, isOutput=False)
    w2 = nc.declare_dram_parameter("w2", [D, D], F32, isOutput=False)
    b2 = nc.declare_dram_parameter("b2", [D], F32, isOutput=False)

    fused = nc.declare_dram_parameter("fused", [L, D], F32, isOutput=True)
    attn = nc.declare_dram_parameter("attn", [L, L], F32, isOutput=True)

    xqT_spill = nc.dram_tensor("xqT_spill", [P, NS, L], F32)


    with tile.TileContext(nc) as tc:
        _build_body(nc, tc, locals())
    nc.compile()
    return nc


def _transpose_in(nc, ptr, x_dram, dstT, ident, xrow_pool):
    """x [L, D] dram -> dstT [P, NS, L] f32r SBUF (feature-major)."""
    xr = _row_tiles(x_dram.ap())
    for ts in range(NS):
        row = xrow_pool.tile([P, D], F32, name="xrow", tag="xrow")
        nc.sync.dma_start(row[:], xr[:, ts, :])
        for ds in range(NS):
            pt = ptr.tile([P, P], F32, name="ptr", tag="ptr", space="PSUM")
            nc.tensor.transpose(pt[:], row[:, ds * P:(ds + 1) * P], ident[:])
            if ds % 2 == 0:
                nc.vector.tensor_copy(dstT[:, ds, ts * P:(ts + 1) * P],
                                      pt[:])
            else:
                nc.scalar.activation(dstT[:, ds, ts * P:(ts + 1) * P],
                                     pt[:], AF.Copy)


def _proj_T(nc, pj, wblk_pool, wrblk_pool, w_dram, b_col, x_T, outT):
    """outT[j, t] = sum_c w[c, j] * xT[c, t] + b[j]; outT [P, NS, L] f32r."""
    wv = w_dram.ap().rearrange("(cs p) j -> p cs j", p=P)
    for js in range(NS):
        wb = wblk_pool.tile([P, NS, P], F32, name="wb", tag="wb")
        nc.sync.dma_start(wb[:], wv[:, :, js * P:(js + 1) * P])
        wr = wrblk_pool.tile([P, NS, P], F32R, name="wr", tag="wr")
        nc.vector.tensor_copy(wr[:], wb[:])
        for ch in range(NCH):
            ps = pj.tile([P, CH], F32, name="pj", tag="pj", space="PSUM")
            for cs in range(NS):
                nc.tensor.matmul(ps[:], wr[:, cs, :],
                                 x_T[:, cs, ch * CH:(ch + 1) * CH],
                                 start=(cs == 0), stop=(cs == NS - 1))
            nc.vector.tensor_scalar(
                outT[:, js, ch * CH:(ch + 1) * CH], ps[:],
                b_col[:, js:js + 1], None, op0=ALU.add)


def _build_body(nc, tc, t):
    xq, xkv = t["xq"], t["xkv"]
    qw, kw, vw, ow = t["qw"], t["kw"], t["vw"], t["ow"]
    qb, kb, vb, ob = t["qb"], t["kb"], t["vb"], t["ob"]
    gg, gbeta, temp = t["gg"], t["gbeta"], t["temp"]
    w1, b1, w2, b2 = t["w1"], t["b1"], t["w2"], t["b2"]
    fused, attn = t["fused"], t["attn"]
    xqT_spill = t["xqT_spill"]

    # ---------- constants (whole-kernel scope) ----------
    s_const = ExitStack()
    const = s_const.enter_context(tc.tile_pool(name="const", bufs=1))

    ident = const.tile([P, P], F32, name="ident")
    make_identity(nc, ident)

    qb_c = const.tile([P, NS], F32, name="qb_c")
    nc.sync.dma_start(qb_c[:], _col_view(qb.ap()))
    kb_c = const.tile([P, NS], F32, name="kb_c")
    nc.sync.dma_start(kb_c[:], _col_view(kb.ap()))
    ob_c = const.tile([P, NS], F32, name="ob_c")
    nc.sync.dma_start(ob_c[:], _col_view(ob.ap()))
    g_c = const.tile([P, NS], F32, name="g_c")
    nc.sync.dma_start(g_c[:], _col_view(gg.ap()))
    beta_c = const.tile([P, NS], F32, name="beta_c")
    nc.sync.dma_start(beta_c[:], _col_view(gbeta.ap()))
    b1_c = const.tile([P, NS], F32, name="b1_c")
    nc.sync.dma_start(b1_c[:], _col_view(b1.ap()))

    # cscale = 1 / (sqrt(HD) * max(temp, 0.1))
    tmp_t = const.tile([P, 1], F32, name="tmp_t")
    nc.sync.dma_start(tmp_t[:], temp.ap()[None, :].to_broadcast([P, 1]))
    nc.vector.tensor_scalar_max(tmp_t[:], tmp_t[:], 0.1)
    nc.vector.tensor_scalar_mul(tmp_t[:], tmp_t[:], float(np.sqrt(HD)))
    cscale = const.tile([P, 1], F32, name="cscale")
    nc.vector.reciprocal(cscale[:], tmp_t[:])

    onesf = const.tile([P, 2], F32, name="onesf")
    nc.vector.memset(onesf[:], 1.0)
    onesr = const.tile([P, 2], F32R, name="onesr")
    nc.vector.tensor_copy(onesr[:], onesf[:])
    eps_t = const.tile([1, 1], F32, name="eps_t")
    nc.vector.memset(eps_t[:], 1e-5)

    # per-head, per-q-subtile softmax denominators (1/rowsum), A->B handoff
    r_all = const.tile([P, H, NS], F32, name="r_all")

    # ---------- ctxT: open early (LIFO), used P2B -> zT matmuls ----------
    s_ctx = ExitStack()
    pool_ctx = s_ctx.enter_context(tc.tile_pool(name="pool_ctx", bufs=1))
    ctxT = pool_ctx.tile([P, NS, L], F32R, name="ctxT")

    # ---------- QT / KT / Vn ----------
    s_qkv = ExitStack()
    pool_qkv = s_qkv.enter_context(tc.tile_pool(name="pool_qkv", bufs=1))
    QT = pool_qkv.tile([P, NS, L], F32R, name="QT")
    KT = pool_qkv.tile([P, NS, L], F32R, name="KT")
    Vn = pool_qkv.tile([P, NS, D], BF16, name="Vn")

    # ================= P1: transposes + projections =================
    s_p1 = ExitStack()
    ptr = s_p1.enter_context(tc.tile_pool(name="ptr", bufs=2, space="PSUM"))
    pj = s_p1.enter_context(tc.tile_pool(name="pj", bufs=3, space="PSUM"))
    xrow_pool = s_p1.enter_context(tc.tile_pool(name="xrow", bufs=2))
    wblk_pool = s_p1.enter_context(tc.tile_pool(name="wblk", bufs=2))
    wrblk_pool = s_p1.enter_context(tc.tile_pool(name="wrblk", bufs=2))

    # xq -> xqT, Q projection, spill xqT (reloaded in P3)
    s_xq = ExitStack()
    pool_xq = s_xq.enter_context(tc.tile_pool(name="pool_xq", bufs=1))
    xqT = pool_xq.tile([P, NS, L], F32R, name="xqT")
    _transpose_in(nc, ptr, xq, xqT, ident, xrow_pool)
    _proj_T(nc, pj, wblk_pool, wrblk_pool, qw, qb_c, xqT, QT)
    nc.sync.dma_start(xqT_spill.ap(), xqT[:].bitcast(F32))
    s_xq.close()

    # xkv -> xkvT, K and V projections
    s_xkv = ExitStack()
    pool_xkv = s_xkv.enter_context(tc.tile_pool(name="pool_xkv", bufs=1))
    xkvT = pool_xkv.tile([P, NS, L], F32R, name="xkvT")
    vb_b = pool_xkv.tile([P, D], F32, name="vb_b")
    nc.sync.dma_start(vb_b[:], vb.ap()[None, :].to_broadcast([P, D]))

    _transpose_in(nc, ptr, xkv, xkvT, ident, xrow_pool)
    _proj_T(nc, pj, wblk_pool, wrblk_pool, kw, kb_c, xkvT, KT)

    # V natural: Vn[t, j] (bf16) = xkvT-blocks (stationary) @ vw (moving)
    s_vw = ExitStack()
    vchunk_pool = s_vw.enter_context(tc.tile_pool(name="vchunk", bufs=1))
    vrchunk_pool = s_vw.enter_context(tc.tile_pool(name="vrchunk", bufs=2))
    vwv = vw.ap().rearrange("(cs p) j -> p cs j", p=P)
    for half in range(4):  # 256-wide moving chunks
        vc = vchunk_pool.tile([P, NS, 256], F32, name="vc", tag="vc")
        nc.sync.dma_start(vc[:], vwv[:, :, half * 256:(half + 1) * 256])
        vr = vrchunk_pool.tile([P, NS, 256], F32R, name="vr", tag="vr")
        nc.vector.tensor_copy(vr[:], vc[:])
        for ts in range(NS):
            ps = pj.tile([P, 256], F32, name="pjv", tag="pjv", space="PSUM")
            for cs in range(NS):
                nc.tensor.matmul(ps[:], xkvT[:, cs, ts * P:(ts + 1) * P],
                                 vr[:, cs, :],
                                 start=(cs == 0), stop=(cs == NS - 1))
            nc.vector.tensor_tensor(
                Vn[:, ts, half * 256:(half + 1) * 256], ps[:],
                vb_b[:, half * 256:(half + 1) * 256], ALU.add)
    s_vw.close()
    s_xkv.close()
    s_p1.close()

    # ================= P2: attention (single merged phase) ==============
    # Per head: S[q,k] matmuls -> exp (bf16 E + rowsum) -> DVE attn-FMA
    # -> DMA-transpose E into ET[k,q] fragments -> aw@V matmuls (bf16).
    # Softmax normalization (1/rowsum) of attn is fused into the FMA; the
    # ctx normalization is deferred to one batched pass at phase end.
    s_A = ExitStack()
    psA = s_A.enter_context(tc.tile_pool(name="psA", bufs=2, space="PSUM"))
    psC = s_A.enter_context(tc.tile_pool(name="psC", bufs=1, space="PSUM"))
    pool_attn = s_A.enter_context(tc.tile_pool(name="pool_attn", bufs=1))
    attn_sb = pool_attn.tile([P, NS, L], F32, name="attn_sb")
    epool = s_A.enter_context(tc.tile_pool(name="epool", bufs=3))
    etpool = s_A.enter_context(tc.tile_pool(name="etpool", bufs=2))
    rsm_pool = s_A.enter_context(tc.tile_pool(name="rsm", bufs=4))

    for h in range(H):
        hs = h // 2
        sub = h % 2
        hp = sub * HD
        qsl = QT[hp:hp + HD, hs, :]
        ksl = KT[hp:hp + HD, hs, :]
        # ET_h[p, qs, ks, q'] = E_h[q', ks*128+p] for q = qs*128+q'
        ET = etpool.tile([P, NS, NS, P], BF16, name="ET", tag="ET")
        ctxp = psC.tile([P, NCH, CH], F32, name=f"ctxp{sub}",
                        tag=f"ctxp{sub}", space="PSUM")
        for qs in range(NS):
            rs_col = rsm_pool.tile([P, 1], F32, name="rs_col", tag="rs_col")
            E = epool.tile([P, L], BF16, name="E", tag="E")
            sA = psA.tile([P, NCH, CH], F32, name="sA", tag="sA",
                          space="PSUM")
            for ch in range(NCH):
                nc.tensor.matmul(sA[:, ch, :], qsl[:, qs * P:(qs + 1) * P],
                                 ksl[:, ch * CH:(ch + 1) * CH],
                                 start=True, stop=True)
            nc.scalar.activation(E[:], sA[:], AF.Exp, scale=cscale[:],
                                 accum_out=rs_col[:])
            nc.vector.reciprocal(r_all[:, h, qs:qs + 1], rs_col[:])
            r16 = rsm_pool.tile([P, 1], F32, name="r16", tag="r16")
            nc.vector.tensor_scalar_mul(r16[:], r_all[:, h, qs:qs + 1],
                                        1.0 / H)
            if h == 0:
                nc.vector.tensor_scalar(attn_sb[:, qs, :], E[:],
                                        r16[:], None, op0=ALU.mult)
            elif qs < 6:
                nc.vector.scalar_tensor_tensor(
                    attn_sb[:, qs, :], E[:], r16[:], attn_sb[:, qs, :],
                    op0=ALU.mult, op1=ALU.add)
            else:
                # offload to GpSimd (no scalar_tensor_tensor there)
                etmp = rsm_pool.tile([P, L], F32, name="etmp", tag="etmp")
                nc.gpsimd.tensor_tensor(etmp[:], E[:],
                                        r16[:].to_broadcast([P, L]),
                                        ALU.mult)
                nc.gpsimd.tensor_tensor(attn_sb[:, qs, :],
                                        attn_sb[:, qs, :], etmp[:], ALU.add)
            nc.sync.dma_start_transpose(ET[:, qs, :, :], E[:])
        # ctx accumulation: lhsT = V columns of this head (stationary),
        # rhs = ET k-rows (strided across qs fragments)
        for ks in range(NS):
            for ch in range(NCH):
                nc.tensor.matmul(
                    ctxp[hp:hp + HD, ch, :],
                    Vn[:, ks, h * HD:(h + 1) * HD],
                    ET[:, 4 * ch:4 * ch + 4, ks, :],
                    start=(ks == 0), stop=(ks == NS - 1))
        for ch in range(NCH):
            nc.vector.tensor_copy(
                ctxT[hp:hp + HD, hs, ch * CH:(ch + 1) * CH],
                ctxp[hp:hp + HD, ch, :])

    nc.sync.dma_start(attn.ap().rearrange("(qs p) k -> p qs k", p=P),
                      attn_sb[:])
    s_A.close()
    s_qkv.close()

    # ---- batched ctx normalization: ctxT[d, q] *= 1/rowsum[h(d), q] ----
    s_rn = ExitStack()
    psR = s_rn.enter_context(tc.tile_pool(name="psR", bufs=1, space="PSUM"))
    rbpool = s_rn.enter_context(tc.tile_pool(name="rb", bufs=2))
    prt = psR.tile([H * NS, P], F32, name="prt", space="PSUM")
    nc.tensor.transpose(prt[:], r_all[:].rearrange("p a b -> p (a b)"),
                        ident[:])
    rT_all = rbpool.tile([H * NS, P], F32, name="rT_all", tag="rT_all")
    nc.vector.tensor_copy(rT_all[:], prt[:])
    for h in range(H):
        hs = h // 2
        hp = (h % 2) * HD
        r_row = rbpool.tile([1, NS, P], F32, name="r_row", tag="r_row")
        nc.sync.dma_start(r_row[:], rT_all[h * NS:(h + 1) * NS, None, :])
        r_b = rbpool.tile([P, L], F32, name="r_b", tag="r_b")
        nc.gpsimd.partition_broadcast(
            r_b[:], r_row[:].rearrange("a b c -> a (b c)"))
        for ch in range(NCH):
            eng = nc.vector if ch == 0 else nc.gpsimd
            eng.tensor_tensor(
                ctxT[hp:hp + HD, hs, ch * CH:(ch + 1) * CH],
                ctxT[hp:hp + HD, hs, ch * CH:(ch + 1) * CH].bitcast(F32),
                r_b[hp:hp + HD, ch * CH:(ch + 1) * CH], ALU.mult)
    s_rn.close()

    # ================= P3: out-proj + LN + fusion =================
    # zT[j, q] = ctx @ ow + ob + xq; after LN applied in place, zT == updT
    s_z = ExitStack()
    pool_z = s_z.enter_context(tc.tile_pool(name="pool_z", bufs=1))
    zT = pool_z.tile([P, NS, L], F32R, name="zT")

    s_xq2 = ExitStack()
    pool_xq2 = s_xq2.enter_context(tc.tile_pool(name="pool_xq2", bufs=1))
    xq2 = pool_xq2.tile([P, NS, L], F32R, name="xq2")
    s_rot = ExitStack()
    xq2_rot = s_rot.enter_context(tc.tile_pool(name="xq2rot", bufs=2))
    for ds in range(NS):
        xr_ = xq2_rot.tile([P, L], F32, name="xq2r", tag="xq2r")
        nc.sync.dma_start(xr_[:], xqT_spill.ap()[:, ds, :])
        nc.vector.tensor_copy(xq2[:, ds, :], xr_[:])
    s_rot.close()

    s_zmm = ExitStack()
    zp_ps = s_zmm.enter_context(tc.tile_pool(name="zp", bufs=2, space="PSUM"))
    stat_ps = s_zmm.enter_context(tc.tile_pool(name="stat", bufs=1,
                                               space="PSUM"))
    s_ow = ExitStack()
    wblk_ow = s_ow.enter_context(tc.tile_pool(name="wblk_ow", bufs=2))
    wrblk_ow = s_ow.enter_context(tc.tile_pool(name="wrblk_ow", bufs=2))
    zsq_pool = s_ow.enter_context(tc.tile_pool(name="zsq", bufs=2))

    owv = ow.ap().rearrange("(cs p) j -> p cs j", p=P)
    s1 = stat_ps.tile([2, NCH, CH], F32, name="s1", tag="s1", space="PSUM")
    s2 = stat_ps.tile([2, NCH, CH], F32, name="s2", tag="s2", space="PSUM")
    for js in range(NS):
        wb = wblk_ow.tile([P, NS, P], F32, name="wb2", tag="wb2")
        nc.sync.dma_start(wb[:], owv[:, :, js * P:(js + 1) * P])
        wr = wrblk_ow.tile([P, NS, P], F32R, name="wr2", tag="wr2")
        nc.vector.tensor_copy(wr[:], wb[:])
        for ch in range(NCH):
            ps = zp_ps.tile([P, CH], F32, name="zpp", tag="zpp", space="PSUM")
            for cs in range(NS):
                nc.tensor.matmul(ps[:], wr[:, cs, :],
                                 ctxT[:, cs, ch * CH:(ch + 1) * CH],
                                 start=(cs == 0), stop=(cs == NS - 1))
            nc.vector.scalar_tensor_tensor(
                zT[:, js, ch * CH:(ch + 1) * CH], ps[:], ob_c[:, js:js + 1],
                xq2[:, js, ch * CH:(ch + 1) * CH].bitcast(F32),
                op0=ALU.add, op1=ALU.add)
        # LN sums over feature dim (partitions) via ones-matmul, accum js
        zsq = zsq_pool.tile([P, L], F32R, name="zsq", tag="zsq")
        nc.gpsimd.tensor_tensor(zsq[:],
                                zT[:, js, :].bitcast(F32),
                                zT[:, js, :].bitcast(F32), ALU.mult)
        for ch in range(NCH):
            nc.tensor.matmul(s1[:, ch, :], onesr[:],
                             zT[:, js, ch * CH:(ch + 1) * CH],
                             start=(js == 0), stop=(js == NS - 1))
            nc.tensor.matmul(s2[:, ch, :], onesr[:],
                             zsq[:, ch * CH:(ch + 1) * CH],
                             start=(js == 0), stop=(js == NS - 1))
    s_ow.close()

    # ---- LN stats rows + partition broadcast ----
    s_stat = ExitStack()
    statrows = s_stat.enter_context(tc.tile_pool(name="statrows", bufs=1))
    mean_r = statrows.tile([1, L], F32, name="mean_r")
    rstd_r = statrows.tile([1, L], F32, name="rstd_r")
    for ch in range(NCH):
        sl = slice(ch * CH, (ch + 1) * CH)
        nc.vector.tensor_scalar_mul(mean_r[:, sl], s1[0:1, ch, :], 1.0 / D)
        nc.vector.tensor_scalar_mul(rstd_r[:, sl], s2[0:1, ch, :], 1.0 / D)
        m2 = statrows.tile([1, CH], F32, name="m2", tag="m2")
        nc.vector.tensor_mul(m2[:], mean_r[:, sl], mean_r[:, sl])
        nc.vector.tensor_sub(rstd_r[:, sl], rstd_r[:, sl], m2[:])
        nc.scalar.activation(rstd_r[:, sl], rstd_r[:, sl], AF.Sqrt,
                             bias=eps_t[:])
        nc.vector.reciprocal(rstd_r[:, sl], rstd_r[:, sl])
    mean_b = statrows.tile([P, L], F32, name="mean_b")
    nc.gpsimd.partition_broadcast(mean_b[:], mean_r[:])
    rstd_b = statrows.tile([P, L], F32, name="rstd_b")
    nc.gpsimd.partition_broadcast(rstd_b[:], rstd_r[:])

    # ---- LN apply, in place over zT (zT becomes updT) ----
    s_lnt = ExitStack()
    ln_tmp_pool = s_lnt.enter_context(tc.tile_pool(name="lntmp", bufs=2))
    for ds in range(NS):
        t1 = ln_tmp_pool.tile([P, L], F32, name="t1", tag="t1")
        nc.vector.tensor_sub(t1[:], zT[:, ds, :].bitcast(F32), mean_b[:])
        nc.vector.scalar_tensor_tensor(t1[:], t1[:], g_c[:, ds:ds + 1],
                                       rstd_b[:], op0=ALU.mult, op1=ALU.mult)
        nc.vector.tensor_scalar(zT[:, ds, :], t1[:],
                                beta_c[:, ds:ds + 1], None, op0=ALU.add)
    s_lnt.close()
    s_stat.close()
    s_zmm.close()

    # ---- fusion mm1: g1T[j, t] = gelu(w1.T @ [updT; xqT] + b1) ----
    s_h1 = ExitStack()
    pool_h1 = s_h1.enter_context(tc.tile_pool(name="pool_h1", bufs=1))
    g1T = pool_h1.tile([P, NS, L], F32R, name="g1T")
    s_w1 = ExitStack()
    h1_ps = s_w1.enter_context(tc.tile_pool(name="h1p", bufs=3, space="PSUM"))
    wblk_w1 = s_w1.enter_context(tc.tile_pool(name="wblk_w1", bufs=2))
    wrblk_w1 = s_w1.enter_context(tc.tile_pool(name="wrblk_w1", bufs=2))
    w1v = w1.ap().rearrange("(cs p) j -> p cs j", p=P)
    for js in range(NS):
        wb = wblk_w1.tile([P, 2 * NS, P], F32, name="wb1", tag="wb1")
        nc.sync.dma_start(wb[:], w1v[:, :, js * P:(js + 1) * P])
        wr = wrblk_w1.tile([P, 2 * NS, P], F32R, name="wr1", tag="wr1")
        nc.vector.tensor_copy(wr[:], wb[:])
        for ch in range(NCH):
            ps = h1_ps.tile([P, CH], F32, name="h1pp", tag="h1pp",
                            space="PSUM")
            for cs in range(2 * NS):
                rhs = (zT[:, cs, ch * CH:(ch + 1) * CH] if cs < NS
                       else xq2[:, cs - NS, ch * CH:(ch + 1) * CH])
                nc.tensor.matmul(ps[:], wr[:, cs, :], rhs,
                                 start=(cs == 0), stop=(cs == 2 * NS - 1))
            nc.scalar.activation(g1T[:, js, ch * CH:(ch + 1) * CH], ps[:],
                                 AF.Gelu, bias=b1_c[:, js:js + 1])
    s_w1.close()

    # ---- fusion mm2: fused[t, j2] = g1T-blocks @ w2 + b2 ----
    s_w2 = ExitStack()
    h2_ps = s_w2.enter_context(tc.tile_pool(name="h2p", bufs=3, space="PSUM"))
    out_pool = s_w2.enter_context(tc.tile_pool(name="outp", bufs=3))
    wblk_w2 = s_w2.enter_context(tc.tile_pool(name="wblk_w2", bufs=1))
    wrblk_w2 = s_w2.enter_context(tc.tile_pool(name="wrblk_w2", bufs=2))
    b2pool = s_w2.enter_context(tc.tile_pool(name="b2pool", bufs=1))
    b2_b = b2pool.tile([P, D], F32, name="b2_b")
    nc.sync.dma_start(b2_b[:], b2.ap()[None, :].to_broadcast([P, D]))

    w2v = w2.ap().rearrange("(js p) j2 -> p js j2", p=P)
    fv = _row_tiles(fused.ap())
    for quad in range(4):  # 256-wide output chunks
        w2c = wblk_w2.tile([P, NS, 256], F32, name="w2c", tag="w2c")
        nc.sync.dma_start(w2c[:], w2v[:, :, quad * 256:(quad + 1) * 256])
        w2r = wrblk_w2.tile([P, NS, 256], F32R, name="w2r", tag="w2r")
        nc.vector.tensor_copy(w2r[:], w2c[:])
        for ts in range(NS):
            ps = h2_ps.tile([P, 256], F32, name="h2pp", tag="h2pp",
                            space="PSUM")
            for js2 in range(NS):
                nc.tensor.matmul(ps[:], g1T[:, js2, ts * P:(ts + 1) * P],
                                 w2r[:, js2, :],
                                 start=(js2 == 0), stop=(js2 == NS - 1))
            orow = out_pool.tile([P, 256], F32, name="orow", tag="orow")
            nc.vector.tensor_tensor(
                orow[:], ps[:], b2_b[:, quad * 256:(quad + 1) * 256],
                ALU.add)
            nc.sync.dma_start(fv[:, ts, quad * 256:(quad + 1) * 256],
                              orow[:])
    s_w2.close()
    s_h1.close()
    s_xq2.close()
    s_z.close()
    s_ctx.close()
    s_const.close()


_NC_CACHE = {}


def _get_program():
    if "nc" not in _NC_CACHE:
        _NC_CACHE["nc"] = build_program()
    return _NC_CACHE["nc"]


def _make_in_maps(inp):
    in_maps = []
    for d in ("s2s", "t2s"):
        qk = "seq_features" if d == "s2s" else "struct_features"
        kk = "struct_features" if d == "s2s" else "seq_features"
        for b in range(4):
            in_maps.append({
                "xq": inp[qk][b], "xkv": inp[kk][b],
                "qw": inp[f"{d}_qw"], "kw": inp[f"{d}_kw"],
                "vw": inp[f"{d}_vw"], "ow": inp[f"{d}_ow"],
                "qb": inp[f"{d}_qb"], "kb": inp[f"{d}_kb"],
                "vb": inp[f"{d}_vb"], "ob": inp[f"{d}_ob"],
                "gg": inp[f"{d}_g"], "gbeta": inp[f"{d}_beta"],
                "temp": inp[f"{d}_temp"],
                "w1": inp["fus_w1"], "b1": inp["fus_b1"],
                "w2": inp["fus_w2"], "b2": inp["fus_b2"],
            })
    return in_maps


def kernel(**inputs):
    inp = {k: np.ascontiguousarray(np.asarray(v), dtype=None)
           for k, v in inputs.items()}
    nc = _get_program()
    in_maps = _make_in_maps(inp)
    res = run_bass_kernel_spmd(nc, in_maps, list(range(8))).results

    seq_fused = np.stack([res[b]["fused"] for b in range(4)])
    struct_fused = np.stack([res[4 + b]["fused"] for b in range(4)])
    s2s_attn = np.stack([res[b]["attn"] for b in range(4)])
    t2s_attn = np.stack([res[4 + b]["attn"] for b in range(4)])
    return seq_fused, struct_fused, s2s_attn, t2s_attn


# revision 32
# speedup vs baseline: 1.0375x; 1.0375x over previous
"""BiDirectionalCrossAttention TRN2 kernel.

Strategy: 8 NeuronCores = 2 attention directions x 4 batch elements, one
(direction, batch) pair per core, zero collectives. Each core runs the full
cross-attention + fusion MLP for its slice.

Layout: activations kept feature-major ("transposed", feature on SBUF
partitions) so every matmul contracts along partitions. Scores computed in
both orientations: [q,k] (softmax reduction + attn-mean output, phase A) and
[k,q] (feeds the aw@V matmul, phase B). Matmuls use fp32r (fp32 with 11-bit
mantissa, full-rate on the PE) except aw@V which uses bf16 (its error is
diluted by the residual path). The attention mask is all-ones per the input
spec and is not applied. Tile pools are opened/closed in strict LIFO order.
"""
import os
import sys

import numpy as np

for _p in ("/opt/trn_rl_repo", "/root/.axon_site/_ro/trn_rl_repo"):
    if os.path.isdir(_p) and _p not in sys.path:
        sys.path.insert(0, _p)

import concourse.bass as bass
import concourse.mybir as mybir
import concourse.tile as tile
from concourse import bacc
from concourse.bass_utils import run_bass_kernel_spmd
from concourse.masks import make_identity
from contextlib import ExitStack

P = 128
L = 1024
D = 1024
H = 16
HD = 64
NS = D // P            # 8 subtiles along a 1024 feature/token dim
CH = 512               # matmul moving-dim chunk (one fp32 PSUM bank)
NCH = L // CH          # 2
F32 = mybir.dt.float32
F32R = mybir.dt.float32r
BF16 = mybir.dt.bfloat16
AF = mybir.ActivationFunctionType
ALU = mybir.AluOpType


def _col_view(ap):
    """[D] dram vector -> [P, NS] (per-partition columns)."""
    return ap.rearrange("(s p) -> p s", p=P)


def _row_tiles(ap):
    """[T, D] dram -> [P, T//P, D] (token-subtile rows)."""
    return ap.rearrange("(ts p) d -> p ts d", p=P)


def build_program():
    nc = bacc.Bacc("TRN2", target_bir_lowering=False, debug=False)

    xq = nc.declare_dram_parameter("xq", [L, D], F32, isOutput=False)
    xkv = nc.declare_dram_parameter("xkv", [L, D], F32, isOutput=False)
    qw = nc.declare_dram_parameter("qw", [D, D], F32, isOutput=False)
    kw = nc.declare_dram_parameter("kw", [D, D], F32, isOutput=False)
    vw = nc.declare_dram_parameter("vw", [D, D], F32, isOutput=False)
    ow = nc.declare_dram_parameter("ow", [D, D], F32, isOutput=False)
    qb = nc.declare_dram_parameter("qb", [D], F32, isOutput=False)
    kb = nc.declare_dram_parameter("kb", [D], F32, isOutput=False)
    vb = nc.declare_dram_parameter("vb", [D], F32, isOutput=False)
    ob = nc.declare_dram_parameter("ob", [D], F32, isOutput=False)
    gg = nc.declare_dram_parameter("gg", [D], F32, isOutput=False)
    gbeta = nc.declare_dram_parameter("gbeta", [D], F32, isOutput=False)
    temp = nc.declare_dram_parameter("temp", [1], F32, isOutput=False)
    w1 = nc.declare_dram_parameter("w1", [2 * D, D], F32, isOutput=False)
    b1 = nc.declare_dram_parameter("b1", [D], F32, isOutput=False)
    w2 = nc.declare_dram_parameter("w2", [D, D], F32, isOutput=False)
    b2 = nc.declare_dram_parameter("b2", [D], F32, isOutput=False)

    fused = nc.declare_dram_parameter("fused", [L, D], F32, isOutput=True)
    attn = nc.declare_dram_parameter("attn", [L, L], F32, isOutput=True)

    xqT_spill = nc.dram_tensor("xqT_spill", [P, NS, L], F32)


    with tile.TileContext(nc) as tc:
        _build_body(nc, tc, locals())
    nc.compile()
    return nc


def _transpose_in(nc, ptr, x_dram, dstT, ident, xrow_pool):
    """x [L, D] dram -> dstT [P, NS, L] f32r SBUF (feature-major)."""
    xr = _row_tiles(x_dram.ap())
    for ts in range(NS):
        row = xrow_pool.tile([P, D], F32, name="xrow", tag="xrow")
        nc.sync.dma_start(row[:], xr[:, ts, :])
        for ds in range(NS):
            pt = ptr.tile([P, P], F32, name="ptr", tag="ptr", space="PSUM")
            nc.tensor.transpose(pt[:], row[:, ds * P:(ds + 1) * P], ident[:])
            if ds % 2 == 0:
                nc.vector.tensor_copy(dstT[:, ds, ts * P:(ts + 1) * P],
                                      pt[:])
            else:
                nc.scalar.activation(dstT[:, ds, ts * P:(ts + 1) * P],
                                     pt[:], AF.Copy)


def _proj_T(nc, pj, wblk_pool, wrblk_pool, w_dram, b_col, x_T, outT):
    """outT[j, t] = sum_c w[c, j] * xT[c, t] + b[j]; outT [P, NS, L] f32r."""
    wv = w_dram.ap().rearrange("(cs p) j -> p cs j", p=P)
    for js in range(NS):
        wb = wblk_pool.tile([P, NS, P], F32, name="wb", tag="wb")
        nc.sync.dma_start(wb[:], wv[:, :, js * P:(js + 1) * P])
        wr = wrblk_pool.tile([P, NS, P], F32R, name="wr", tag="wr")
        nc.vector.tensor_copy(wr[:], wb[:])
        for ch in range(NCH):
            ps = pj.tile([P, CH], F32, name="pj", tag="pj", space="PSUM")
            for cs in range(NS):
                nc.tensor.matmul(ps[:], wr[:, cs, :],
                                 x_T[:, cs, ch * CH:(ch + 1) * CH],
                                 start=(cs == 0), stop=(cs == NS - 1))
            nc.vector.tensor_scalar(
                outT[:, js, ch * CH:(ch + 1) * CH], ps[:],
                b_col[:, js:js + 1], None, op0=ALU.add)


def _build_body(nc, tc, t):
    xq, xkv = t["xq"], t["xkv"]
    qw, kw, vw, ow = t["qw"], t["kw"], t["vw"], t["ow"]
    qb, kb, vb, ob = t["qb"], t["kb"], t["vb"], t["ob"]
    gg, gbeta, temp = t["gg"], t["gbeta"], t["temp"]
    w1, b1, w2, b2 = t["w1"], t["b1"], t["w2"], t["b2"]
    fused, attn = t["fused"], t["attn"]
    xqT_spill = t["xqT_spill"]

    # ---------- constants (whole-kernel scope) ----------
    s_const = ExitStack()
    const = s_const.enter_context(tc.tile_pool(name="const", bufs=1))

    ident = const.tile([P, P], F32, name="ident")
    make_identity(nc, ident)

    qb_c = const.tile([P, NS], F32, name="qb_c")
    nc.sync.dma_start(qb_c[:], _col_view(qb.ap()))
    kb_c = const.tile([P, NS], F32, name="kb_c")
    nc.sync.dma_start(kb_c[:], _col_view(kb.ap()))
    ob_c = const.tile([P, NS], F32, name="ob_c")
    nc.sync.dma_start(ob_c[:], _col_view(ob.ap()))
    g_c = const.tile([P, NS], F32, name="g_c")
    nc.sync.dma_start(g_c[:], _col_view(gg.ap()))
    beta_c = const.tile([P, NS], F32, name="beta_c")
    nc.sync.dma_start(beta_c[:], _col_view(gbeta.ap()))
    b1_c = const.tile([P, NS], F32, name="b1_c")
    nc.sync.dma_start(b1_c[:], _col_view(b1.ap()))

    # cscale = 1 / (sqrt(HD) * max(temp, 0.1))
    tmp_t = const.tile([P, 1], F32, name="tmp_t")
    nc.sync.dma_start(tmp_t[:], temp.ap()[None, :].to_broadcast([P, 1]))
    nc.vector.tensor_scalar_max(tmp_t[:], tmp_t[:], 0.1)
    nc.vector.tensor_scalar_mul(tmp_t[:], tmp_t[:], float(np.sqrt(HD)))
    cscale = const.tile([P, 1], F32, name="cscale")
    nc.vector.reciprocal(cscale[:], tmp_t[:])

    onesf = const.tile([P, 2], F32, name="onesf")
    nc.vector.memset(onesf[:], 1.0)
    onesr = const.tile([P, 2], F32R, name="onesr")
    nc.vector.tensor_copy(onesr[:], onesf[:])
    eps_t = const.tile([1, 1], F32, name="eps_t")
    nc.vector.memset(eps_t[:], 1e-5)

    # per-head, per-q-subtile softmax denominators (1/rowsum), A->B handoff
    r_all = const.tile([P, H, NS], F32, name="r_all")

    # ---------- ctxT: open early (LIFO), used P2B -> zT matmuls ----------
    s_ctx = ExitStack()
    pool_ctx = s_ctx.enter_context(tc.tile_pool(name="pool_ctx", bufs=1))
    ctxT = pool_ctx.tile([P, NS, L], F32R, name="ctxT")

    # ---------- QT / KT / Vn ----------
    s_qkv = ExitStack()
    pool_qkv = s_qkv.enter_context(tc.tile_pool(name="pool_qkv", bufs=1))
    QT = pool_qkv.tile([P, NS, L], F32R, name="QT")
    KT = pool_qkv.tile([P, NS, L], F32R, name="KT")
    Vn = pool_qkv.tile([P, NS, D], BF16, name="Vn")

    # ================= P1: transposes + projections =================
    s_p1 = ExitStack()
    ptr = s_p1.enter_context(tc.tile_pool(name="ptr", bufs=2, space="PSUM"))
    pj = s_p1.enter_context(tc.tile_pool(name="pj", bufs=3, space="PSUM"))
    xrow_pool = s_p1.enter_context(tc.tile_pool(name="xrow", bufs=2))
    wblk_pool = s_p1.enter_context(tc.tile_pool(name="wblk", bufs=2))
    wrblk_pool = s_p1.enter_context(tc.tile_pool(name="wrblk", bufs=2))

    # xq -> xqT, Q projection, spill xqT (reloaded in P3)
    s_xq = ExitStack()
    pool_xq = s_xq.enter_context(tc.tile_pool(name="pool_xq", bufs=1))
    xqT = pool_xq.tile([P, NS, L], F32R, name="xqT")
    _transpose_in(nc, ptr, xq, xqT, ident, xrow_pool)
    _proj_T(nc, pj, wblk_pool, wrblk_pool, qw, qb_c, xqT, QT)
    nc.sync.dma_start(xqT_spill.ap(), xqT[:].bitcast(F32))
    s_xq.close()

    # xkv -> xkvT, K and V projections
    s_xkv = ExitStack()
    pool_xkv = s_xkv.enter_context(tc.tile_pool(name="pool_xkv", bufs=1))
    xkvT = pool_xkv.tile([P, NS, L], F32R, name="xkvT")
    vb_b = pool_xkv.tile([P, D], F32, name="vb_b")
    nc.sync.dma_start(vb_b[:], vb.ap()[None, :].to_broadcast([P, D]))

    _transpose_in(nc, ptr, xkv, xkvT, ident, xrow_pool)
    _proj_T(nc, pj, wblk_pool, wrblk_pool, kw, kb_c, xkvT, KT)

    # V natural: Vn[t, j] (bf16) = xkvT-blocks (stationary) @ vw (moving)
    s_vw = ExitStack()
    vchunk_pool = s_vw.enter_context(tc.tile_pool(name="vchunk", bufs=1))
    vrchunk_pool = s_vw.enter_context(tc.tile_pool(name="vrchunk", bufs=2))
    vwv = vw.ap().rearrange("(cs p) j -> p cs j", p=P)
    for half in range(4):  # 256-wide moving chunks
        vc = vchunk_pool.tile([P, NS, 256], F32, name="vc", tag="vc")
        nc.sync.dma_start(vc[:], vwv[:, :, half * 256:(half + 1) * 256])
        vr = vrchunk_pool.tile([P, NS, 256], F32R, name="vr", tag="vr")
        nc.vector.tensor_copy(vr[:], vc[:])
        for ts in range(NS):
            ps = pj.tile([P, 256], F32, name="pjv", tag="pjv", space="PSUM")
            for cs in range(NS):
                nc.tensor.matmul(ps[:], xkvT[:, cs, ts * P:(ts + 1) * P],
                                 vr[:, cs, :],
                                 start=(cs == 0), stop=(cs == NS - 1))
            nc.vector.tensor_tensor(
                Vn[:, ts, half * 256:(half + 1) * 256], ps[:],
                vb_b[:, half * 256:(half + 1) * 256], ALU.add)
    s_vw.close()
    s_xkv.close()
    s_p1.close()

    # ================= P2: attention (single merged phase) ==============
    # Per head: S[q,k] matmuls -> exp (bf16 E + rowsum) -> DVE attn-FMA
    # -> DMA-transpose E into ET[k,q] fragments -> aw@V matmuls (bf16).
    # Softmax normalization (1/rowsum) of attn is fused into the FMA; the
    # ctx normalization is deferred to one batched pass at phase end.
    s_A = ExitStack()
    psA = s_A.enter_context(tc.tile_pool(name="psA", bufs=2, space="PSUM"))
    psC = s_A.enter_context(tc.tile_pool(name="psC", bufs=1, space="PSUM"))
    pool_attn = s_A.enter_context(tc.tile_pool(name="pool_attn", bufs=1))
    attn_sb = pool_attn.tile([P, NS, L], F32, name="attn_sb")
    epool = s_A.enter_context(tc.tile_pool(name="epool", bufs=3))
    etpool = s_A.enter_context(tc.tile_pool(name="etpool", bufs=2))
    rsm_pool = s_A.enter_context(tc.tile_pool(name="rsm", bufs=4))

    for h in range(H):
        hs = h // 2
        sub = h % 2
        hp = sub * HD
        qsl = QT[hp:hp + HD, hs, :]
        ksl = KT[hp:hp + HD, hs, :]
        # ET_h[p, qs, ks, q'] = E_h[q', ks*128+p] for q = qs*128+q'
        ET = etpool.tile([P, NS, NS, P], BF16, name="ET", tag="ET")
        ctxp = psC.tile([P, NCH, CH], F32, name=f"ctxp{sub}",
                        tag=f"ctxp{sub}", space="PSUM")
        for qs in range(NS):
            rs_col = rsm_pool.tile([P, 1], F32, name="rs_col", tag="rs_col")
            E = epool.tile([P, L], BF16, name="E", tag="E")
            sA = psA.tile([P, NCH, CH], F32, name="sA", tag="sA",
                          space="PSUM")
            for ch in range(NCH):
                nc.tensor.matmul(sA[:, ch, :], qsl[:, qs * P:(qs + 1) * P],
                                 ksl[:, ch * CH:(ch + 1) * CH],
                                 start=True, stop=True)
            nc.scalar.activation(E[:], sA[:], AF.Exp, scale=cscale[:],
                                 accum_out=rs_col[:])
            nc.vector.reciprocal(r_all[:, h, qs:qs + 1], rs_col[:])
            r16 = rsm_pool.tile([P, 1], F32, name="r16", tag="r16")
            nc.vector.tensor_scalar_mul(r16[:], r_all[:, h, qs:qs + 1],
                                        1.0 / H)
            if h == 0:
                nc.vector.tensor_scalar(attn_sb[:, qs, :], E[:],
                                        r16[:], None, op0=ALU.mult)
            else:
                nc.vector.scalar_tensor_tensor(
                    attn_sb[:, qs, :], E[:], r16[:], attn_sb[:, qs, :],
                    op0=ALU.mult, op1=ALU.add)
            nc.sync.dma_start_transpose(ET[:, qs, :, :], E[:])
        # ctx accumulation: lhsT = V columns of this head (stationary),
        # rhs = ET k-rows (strided across qs fragments)
        for ks in range(NS):
            for ch in range(NCH):
                nc.tensor.matmul(
                    ctxp[hp:hp + HD, ch, :],
                    Vn[:, ks, h * HD:(h + 1) * HD],
                    ET[:, 4 * ch:4 * ch + 4, ks, :],
                    start=(ks == 0), stop=(ks == NS - 1))
        for ch in range(NCH):
            nc.vector.tensor_copy(
                ctxT[hp:hp + HD, hs, ch * CH:(ch + 1) * CH],
                ctxp[hp:hp + HD, ch, :])

    nc.sync.dma_start(attn.ap().rearrange("(qs p) k -> p qs k", p=P),
                      attn_sb[:])
    s_A.close()
    s_qkv.close()

    # ---- batched ctx normalization: ctxT[d, q] *= 1/rowsum[h(d), q] ----
    s_rn = ExitStack()
    psR = s_rn.enter_context(tc.tile_pool(name="psR", bufs=1, space="PSUM"))
    rbpool = s_rn.enter_context(tc.tile_pool(name="rb", bufs=2))
    prt = psR.tile([H * NS, P], F32, name="prt", space="PSUM")
    nc.tensor.transpose(prt[:], r_all[:].rearrange("p a b -> p (a b)"),
                        ident[:])
    rT_all = rbpool.tile([H * NS, P], F32, name="rT_all", tag="rT_all")
    nc.vector.tensor_copy(rT_all[:], prt[:])
    for h in range(H):
        hs = h // 2
        hp = (h % 2) * HD
        r_row = rbpool.tile([1, NS, P], F32, name="r_row", tag="r_row")
        nc.sync.dma_start(r_row[:], rT_all[h * NS:(h + 1) * NS, None, :])
        r_b = rbpool.tile([P, L], F32, name="r_b", tag="r_b")
        nc.gpsimd.partition_broadcast(
            r_b[:], r_row[:].rearrange("a b c -> a (b c)"))
        for ch in range(NCH):
            eng = nc.vector if ch == 0 else nc.gpsimd
            eng.tensor_tensor(
                ctxT[hp:hp + HD, hs, ch * CH:(ch + 1) * CH],
                ctxT[hp:hp + HD, hs, ch * CH:(ch + 1) * CH].bitcast(F32),
                r_b[hp:hp + HD, ch * CH:(ch + 1) * CH], ALU.mult)
    s_rn.close()

    # ================= P3: out-proj + LN + fusion =================
    # zT[j, q] = ctx @ ow + ob + xq; after LN applied in place, zT == updT
    s_z = ExitStack()
    pool_z = s_z.enter_context(tc.tile_pool(name="pool_z", bufs=1))
    zT = pool_z.tile([P, NS, L], F32R, name="zT")

    s_xq2 = ExitStack()
    pool_xq2 = s_xq2.enter_context(tc.tile_pool(name="pool_xq2", bufs=1))
    xq2 = pool_xq2.tile([P, NS, L], F32R, name="xq2")
    s_rot = ExitStack()
    xq2_rot = s_rot.enter_context(tc.tile_pool(name="xq2rot", bufs=2))
    for ds in range(NS):
        xr_ = xq2_rot.tile([P, L], F32, name="xq2r", tag="xq2r")
        nc.sync.dma_start(xr_[:], xqT_spill.ap()[:, ds, :])
        nc.vector.tensor_copy(xq2[:, ds, :], xr_[:])
    s_rot.close()

    s_zmm = ExitStack()
    zp_ps = s_zmm.enter_context(tc.tile_pool(name="zp", bufs=2, space="PSUM"))
    stat_ps = s_zmm.enter_context(tc.tile_pool(name="stat", bufs=1,
                                               space="PSUM"))
    s_ow = ExitStack()
    wblk_ow = s_ow.enter_context(tc.tile_pool(name="wblk_ow", bufs=2))
    wrblk_ow = s_ow.enter_context(tc.tile_pool(name="wrblk_ow", bufs=2))
    zsq_pool = s_ow.enter_context(tc.tile_pool(name="zsq", bufs=2))

    owv = ow.ap().rearrange("(cs p) j -> p cs j", p=P)
    s1 = stat_ps.tile([2, NCH, CH], F32, name="s1", tag="s1", space="PSUM")
    s2 = stat_ps.tile([2, NCH, CH], F32, name="s2", tag="s2", space="PSUM")
    for js in range(NS):
        wb = wblk_ow.tile([P, NS, P], F32, name="wb2", tag="wb2")
        nc.sync.dma_start(wb[:], owv[:, :, js * P:(js + 1) * P])
        wr = wrblk_ow.tile([P, NS, P], F32R, name="wr2", tag="wr2")
        nc.vector.tensor_copy(wr[:], wb[:])
        for ch in range(NCH):
            ps = zp_ps.tile([P, CH], F32, name="zpp", tag="zpp", space="PSUM")
            for cs in range(NS):
                nc.tensor.matmul(ps[:], wr[:, cs, :],
                                 ctxT[:, cs, ch * CH:(ch + 1) * CH],
                                 start=(cs == 0), stop=(cs == NS - 1))
            nc.vector.scalar_tensor_tensor(
                zT[:, js, ch * CH:(ch + 1) * CH], ps[:], ob_c[:, js:js + 1],
                xq2[:, js, ch * CH:(ch + 1) * CH].bitcast(F32),
                op0=ALU.add, op1=ALU.add)
        # LN sums over feature dim (partitions) via ones-matmul, accum js
        zsq = zsq_pool.tile([P, L], F32R, name="zsq", tag="zsq")
        nc.gpsimd.tensor_tensor(zsq[:],
                                zT[:, js, :].bitcast(F32),
                                zT[:, js, :].bitcast(F32), ALU.mult)
        for ch in range(NCH):
            nc.tensor.matmul(s1[:, ch, :], onesr[:],
                             zT[:, js, ch * CH:(ch + 1) * CH],
                             start=(js == 0), stop=(js == NS - 1))
            nc.tensor.matmul(s2[:, ch, :], onesr[:],
                             zsq[:, ch * CH:(ch + 1) * CH],
                             start=(js == 0), stop=(js == NS - 1))
    s_ow.close()

    # ---- LN stats rows + partition broadcast ----
    s_stat = ExitStack()
    statrows = s_stat.enter_context(tc.tile_pool(name="statrows", bufs=1))
    mean_r = statrows.tile([1, L], F32, name="mean_r")
    rstd_r = statrows.tile([1, L], F32, name="rstd_r")
    for ch in range(NCH):
        sl = slice(ch * CH, (ch + 1) * CH)
        nc.vector.tensor_scalar_mul(mean_r[:, sl], s1[0:1, ch, :], 1.0 / D)
        nc.vector.tensor_scalar_mul(rstd_r[:, sl], s2[0:1, ch, :], 1.0 / D)
        m2 = statrows.tile([1, CH], F32, name="m2", tag="m2")
        nc.vector.tensor_mul(m2[:], mean_r[:, sl], mean_r[:, sl])
        nc.vector.tensor_sub(rstd_r[:, sl], rstd_r[:, sl], m2[:])
        nc.scalar.activation(rstd_r[:, sl], rstd_r[:, sl], AF.Sqrt,
                             bias=eps_t[:])
        nc.vector.reciprocal(rstd_r[:, sl], rstd_r[:, sl])
    mean_b = statrows.tile([P, L], F32, name="mean_b")
    nc.gpsimd.partition_broadcast(mean_b[:], mean_r[:])
    rstd_b = statrows.tile([P, L], F32, name="rstd_b")
    nc.gpsimd.partition_broadcast(rstd_b[:], rstd_r[:])

    # ---- LN apply, in place over zT (zT becomes updT) ----
    s_lnt = ExitStack()
    ln_tmp_pool = s_lnt.enter_context(tc.tile_pool(name="lntmp", bufs=2))
    for ds in range(NS):
        t1 = ln_tmp_pool.tile([P, L], F32, name="t1", tag="t1")
        nc.vector.tensor_sub(t1[:], zT[:, ds, :].bitcast(F32), mean_b[:])
        nc.vector.scalar_tensor_tensor(t1[:], t1[:], g_c[:, ds:ds + 1],
                                       rstd_b[:], op0=ALU.mult, op1=ALU.mult)
        nc.vector.tensor_scalar(zT[:, ds, :], t1[:],
                                beta_c[:, ds:ds + 1], None, op0=ALU.add)
    s_lnt.close()
    s_stat.close()
    s_zmm.close()

    # ---- fusion mm1: g1T[j, t] = gelu(w1.T @ [updT; xqT] + b1) ----
    s_h1 = ExitStack()
    pool_h1 = s_h1.enter_context(tc.tile_pool(name="pool_h1", bufs=1))
    g1T = pool_h1.tile([P, NS, L], F32R, name="g1T")
    s_w1 = ExitStack()
    h1_ps = s_w1.enter_context(tc.tile_pool(name="h1p", bufs=3, space="PSUM"))
    wblk_w1 = s_w1.enter_context(tc.tile_pool(name="wblk_w1", bufs=2))
    wrblk_w1 = s_w1.enter_context(tc.tile_pool(name="wrblk_w1", bufs=2))
    w1v = w1.ap().rearrange("(cs p) j -> p cs j", p=P)
    for js in range(NS):
        wb = wblk_w1.tile([P, 2 * NS, P], F32, name="wb1", tag="wb1")
        nc.sync.dma_start(wb[:], w1v[:, :, js * P:(js + 1) * P])
        wr = wrblk_w1.tile([P, 2 * NS, P], F32R, name="wr1", tag="wr1")
        nc.vector.tensor_copy(wr[:], wb[:])
        for ch in range(NCH):
            ps = h1_ps.tile([P, CH], F32, name="h1pp", tag="h1pp",
                            space="PSUM")
            for cs in range(2 * NS):
                rhs = (zT[:, cs, ch * CH:(ch + 1) * CH] if cs < NS
                       else xq2[:, cs - NS, ch * CH:(ch + 1) * CH])
                nc.tensor.matmul(ps[:], wr[:, cs, :], rhs,
                                 start=(cs == 0), stop=(cs == 2 * NS - 1))
            nc.scalar.activation(g1T[:, js, ch * CH:(ch + 1) * CH], ps[:],
                                 AF.Gelu, bias=b1_c[:, js:js + 1])
    s_w1.close()

    # ---- fusion mm2: fused[t, j2] = g1T-blocks @ w2 + b2 ----
    s_w2 = ExitStack()
    h2_ps = s_w2.enter_context(tc.tile_pool(name="h2p", bufs=3, space="PSUM"))
    out_pool = s_w2.enter_context(tc.tile_pool(name="outp", bufs=3))
    wblk_w2 = s_w2.enter_context(tc.tile_pool(name="wblk_w2", bufs=1))
    wrblk_w2 = s_w2.enter_context(tc.tile_pool(name="wrblk_w2", bufs=2))
    b2pool = s_w2.enter_context(tc.tile_pool(name="b2pool", bufs=1))
    b2_b = b2pool.tile([P, D], F32, name="b2_b")
    nc.sync.dma_start(b2_b[:], b2.ap()[None, :].to_broadcast([P, D]))

    w2v = w2.ap().rearrange("(js p) j2 -> p js j2", p=P)
    fv = _row_tiles(fused.ap())
    for quad in range(4):  # 256-wide output chunks
        w2c = wblk_w2.tile([P, NS, 256], F32, name="w2c", tag="w2c")
        nc.sync.dma_start(w2c[:], w2v[:, :, quad * 256:(quad + 1) * 256])
        w2r = wrblk_w2.tile([P, NS, 256], F32R, name="w2r", tag="w2r")
        nc.vector.tensor_copy(w2r[:], w2c[:])
        for ts in range(NS):
            ps = h2_ps.tile([P, 256], F32, name="h2pp", tag="h2pp",
                            space="PSUM")
            for js2 in range(NS):
                nc.tensor.matmul(ps[:], g1T[:, js2, ts * P:(ts + 1) * P],
                                 w2r[:, js2, :],
                                 start=(js2 == 0), stop=(js2 == NS - 1))
            orow = out_pool.tile([P, 256], F32, name="orow", tag="orow")
            nc.vector.tensor_tensor(
                orow[:], ps[:], b2_b[:, quad * 256:(quad + 1) * 256],
                ALU.add)
            nc.sync.dma_start(fv[:, ts, quad * 256:(quad + 1) * 256],
                              orow[:])
    s_w2.close()
    s_h1.close()
    s_xq2.close()
    s_z.close()
    s_ctx.close()
    s_const.close()


_NC_CACHE = {}


def _get_program():
    if "nc" not in _NC_CACHE:
        _NC_CACHE["nc"] = build_program()
    return _NC_CACHE["nc"]


def _make_in_maps(inp):
    in_maps = []
    for d in ("s2s", "t2s"):
        qk = "seq_features" if d == "s2s" else "struct_features"
        kk = "struct_features" if d == "s2s" else "seq_features"
        for b in range(4):
            in_maps.append({
                "xq": inp[qk][b], "xkv": inp[kk][b],
                "qw": inp[f"{d}_qw"], "kw": inp[f"{d}_kw"],
                "vw": inp[f"{d}_vw"], "ow": inp[f"{d}_ow"],
                "qb": inp[f"{d}_qb"], "kb": inp[f"{d}_kb"],
                "vb": inp[f"{d}_vb"], "ob": inp[f"{d}_ob"],
                "gg": inp[f"{d}_g"], "gbeta": inp[f"{d}_beta"],
                "temp": inp[f"{d}_temp"],
                "w1": inp["fus_w1"], "b1": inp["fus_b1"],
                "w2": inp["fus_w2"], "b2": inp["fus_b2"],
            })
    return in_maps


def kernel(**inputs):
    inp = {k: np.ascontiguousarray(np.asarray(v), dtype=None)
           for k, v in inputs.items()}
    nc = _get_program()
    in_maps = _make_in_maps(inp)
    res = run_bass_kernel_spmd(nc, in_maps, list(range(8))).results

    seq_fused = np.stack([res[b]["fused"] for b in range(4)])
    struct_fused = np.stack([res[4 + b]["fused"] for b in range(4)])
    s2s_attn = np.stack([res[b]["attn"] for b in range(4)])
    t2s_attn = np.stack([res[4 + b]["attn"] for b in range(4)])
    return seq_fused, struct_fused, s2s_attn, t2s_attn


# revision 33
# speedup vs baseline: 1.3396x; 1.2911x over previous
"""BiDirectionalCrossAttention TRN2 kernel.

Strategy: 8 NeuronCores = 2 attention directions x 4 batch elements, one
(direction, batch) pair per core, zero collectives. Each core runs the full
cross-attention + fusion MLP for its slice.

Layout: activations kept feature-major ("transposed", feature on SBUF
partitions) so every matmul contracts along partitions. Scores computed in
both orientations: [q,k] (softmax reduction + attn-mean output, phase A) and
[k,q] (feeds the aw@V matmul, phase B). Matmuls use fp32r (fp32 with 11-bit
mantissa, full-rate on the PE) except aw@V which uses bf16 (its error is
diluted by the residual path). The attention mask is all-ones per the input
spec and is not applied. Tile pools are opened/closed in strict LIFO order.
"""
import os
import sys

import numpy as np

for _p in ("/opt/trn_rl_repo", "/root/.axon_site/_ro/trn_rl_repo"):
    if os.path.isdir(_p) and _p not in sys.path:
        sys.path.insert(0, _p)

import concourse.bass as bass
import concourse.mybir as mybir
import concourse.tile as tile
from concourse import bacc
from concourse.bass_utils import run_bass_kernel_spmd
from concourse.masks import make_identity
from contextlib import ExitStack

P = 128
L = 1024
D = 1024
H = 16
HD = 64
NS = D // P            # 8 subtiles along a 1024 feature/token dim
CH = 512               # matmul moving-dim chunk (one fp32 PSUM bank)
NCH = L // CH          # 2
F32 = mybir.dt.float32
F32R = mybir.dt.float32r
BF16 = mybir.dt.bfloat16
AF = mybir.ActivationFunctionType
ALU = mybir.AluOpType


def _col_view(ap):
    """[D] dram vector -> [P, NS] (per-partition columns)."""
    return ap.rearrange("(s p) -> p s", p=P)


def _row_tiles(ap):
    """[T, D] dram -> [P, T//P, D] (token-subtile rows)."""
    return ap.rearrange("(ts p) d -> p ts d", p=P)


def build_program():
    nc = bacc.Bacc("TRN2", target_bir_lowering=False, debug=False)

    xq = nc.declare_dram_parameter("xq", [L, D], F32, isOutput=False)
    xkv = nc.declare_dram_parameter("xkv", [L, D], F32, isOutput=False)
    qw = nc.declare_dram_parameter("qw", [D, D], F32, isOutput=False)
    kw = nc.declare_dram_parameter("kw", [D, D], F32, isOutput=False)
    vw = nc.declare_dram_parameter("vw", [D, D], F32, isOutput=False)
    ow = nc.declare_dram_parameter("ow", [D, D], F32, isOutput=False)
    qb = nc.declare_dram_parameter("qb", [D], F32, isOutput=False)
    kb = nc.declare_dram_parameter("kb", [D], F32, isOutput=False)
    vb = nc.declare_dram_parameter("vb", [D], F32, isOutput=False)
    ob = nc.declare_dram_parameter("ob", [D], F32, isOutput=False)
    gg = nc.declare_dram_parameter("gg", [D], F32, isOutput=False)
    gbeta = nc.declare_dram_parameter("gbeta", [D], F32, isOutput=False)
    temp = nc.declare_dram_parameter("temp", [1], F32, isOutput=False)
    w1 = nc.declare_dram_parameter("w1", [2 * D, D], F32, isOutput=False)
    b1 = nc.declare_dram_parameter("b1", [D], F32, isOutput=False)
    w2 = nc.declare_dram_parameter("w2", [D, D], F32, isOutput=False)
    b2 = nc.declare_dram_parameter("b2", [D], F32, isOutput=False)

    fused = nc.declare_dram_parameter("fused", [L, D], F32, isOutput=True)
    attn = nc.declare_dram_parameter("attn", [L, L], F32, isOutput=True)

    xqT_spill = nc.dram_tensor("xqT_spill", [P, NS, L], F32)


    with tile.TileContext(nc) as tc:
        _build_body(nc, tc, locals())
    nc.compile()
    return nc


def _transpose_in(nc, ptr, x_dram, dstT, ident, xrow_pool):
    """x [L, D] dram -> dstT [P, NS, L] f32r SBUF (feature-major)."""
    xr = _row_tiles(x_dram.ap())
    for ts in range(NS):
        row = xrow_pool.tile([P, D], F32, name="xrow", tag="xrow")
        nc.sync.dma_start(row[:], xr[:, ts, :])
        for ds in range(NS):
            pt = ptr.tile([P, P], F32, name="ptr", tag="ptr", space="PSUM")
            nc.tensor.transpose(pt[:], row[:, ds * P:(ds + 1) * P], ident[:])
            if ds % 2 == 0:
                nc.vector.tensor_copy(dstT[:, ds, ts * P:(ts + 1) * P],
                                      pt[:])
            else:
                nc.scalar.activation(dstT[:, ds, ts * P:(ts + 1) * P],
                                     pt[:], AF.Copy)


def _proj_T(nc, pj, wblk_pool, wrblk_pool, w_dram, b_col, x_T, outT):
    """outT[j, t] = sum_c w[c, j] * xT[c, t] + b[j]; outT [P, NS, L] f32r."""
    wv = w_dram.ap().rearrange("(cs p) j -> p cs j", p=P)
    for js in range(NS):
        wb = wblk_pool.tile([P, NS, P], F32, name="wb", tag="wb")
        nc.sync.dma_start(wb[:], wv[:, :, js * P:(js + 1) * P])
        wr = wrblk_pool.tile([P, NS, P], F32R, name="wr", tag="wr")
        nc.vector.tensor_copy(wr[:], wb[:])
        for ch in range(NCH):
            ps = pj.tile([P, CH], F32, name="pj", tag="pj", space="PSUM")
            for cs in range(NS):
                nc.tensor.matmul(ps[:], wr[:, cs, :],
                                 x_T[:, cs, ch * CH:(ch + 1) * CH],
                                 start=(cs == 0), stop=(cs == NS - 1))
            nc.vector.tensor_scalar(
                outT[:, js, ch * CH:(ch + 1) * CH], ps[:],
                b_col[:, js:js + 1], None, op0=ALU.add)


def _build_body(nc, tc, t):
    xq, xkv = t["xq"], t["xkv"]
    qw, kw, vw, ow = t["qw"], t["kw"], t["vw"], t["ow"]
    qb, kb, vb, ob = t["qb"], t["kb"], t["vb"], t["ob"]
    gg, gbeta, temp = t["gg"], t["gbeta"], t["temp"]
    w1, b1, w2, b2 = t["w1"], t["b1"], t["w2"], t["b2"]
    fused, attn = t["fused"], t["attn"]
    xqT_spill = t["xqT_spill"]

    # ---------- constants (whole-kernel scope) ----------
    s_const = ExitStack()
    const = s_const.enter_context(tc.tile_pool(name="const", bufs=1))

    ident = const.tile([P, P], F32, name="ident")
    make_identity(nc, ident)

    qb_c = const.tile([P, NS], F32, name="qb_c")
    nc.sync.dma_start(qb_c[:], _col_view(qb.ap()))
    kb_c = const.tile([P, NS], F32, name="kb_c")
    nc.sync.dma_start(kb_c[:], _col_view(kb.ap()))
    ob_c = const.tile([P, NS], F32, name="ob_c")
    nc.sync.dma_start(ob_c[:], _col_view(ob.ap()))
    g_c = const.tile([P, NS], F32, name="g_c")
    nc.sync.dma_start(g_c[:], _col_view(gg.ap()))
    beta_c = const.tile([P, NS], F32, name="beta_c")
    nc.sync.dma_start(beta_c[:], _col_view(gbeta.ap()))
    b1_c = const.tile([P, NS], F32, name="b1_c")
    nc.sync.dma_start(b1_c[:], _col_view(b1.ap()))

    # cscale = 1 / (sqrt(HD) * max(temp, 0.1))
    tmp_t = const.tile([P, 1], F32, name="tmp_t")
    nc.sync.dma_start(tmp_t[:], temp.ap()[None, :].to_broadcast([P, 1]))
    nc.vector.tensor_scalar_max(tmp_t[:], tmp_t[:], 0.1)
    nc.vector.tensor_scalar_mul(tmp_t[:], tmp_t[:], float(np.sqrt(HD)))
    cscale = const.tile([P, 1], F32, name="cscale")
    nc.vector.reciprocal(cscale[:], tmp_t[:])

    onesf = const.tile([P, 2], F32, name="onesf")
    nc.vector.memset(onesf[:], 1.0)
    onesr = const.tile([P, 2], F32R, name="onesr")
    nc.vector.tensor_copy(onesr[:], onesf[:])
    eps_t = const.tile([1, 1], F32, name="eps_t")
    nc.vector.memset(eps_t[:], 1e-5)

    # per-head, per-q-subtile softmax denominators (1/rowsum), A->B handoff
    r_all = const.tile([P, H, NS], F32, name="r_all")

    # ---------- ctxT: open early (LIFO), used P2B -> zT matmuls ----------
    s_ctx = ExitStack()
    pool_ctx = s_ctx.enter_context(tc.tile_pool(name="pool_ctx", bufs=1))
    ctxT = pool_ctx.tile([P, NS, L], F32R, name="ctxT")

    # ---------- QT / KT / Vn ----------
    s_qkv = ExitStack()
    pool_qkv = s_qkv.enter_context(tc.tile_pool(name="pool_qkv", bufs=1))
    QT = pool_qkv.tile([P, NS, L], F32R, name="QT")
    KT = pool_qkv.tile([P, NS, L], F32R, name="KT")
    Vn = pool_qkv.tile([P, NS, D], BF16, name="Vn")

    # ================= P1: transposes + projections =================
    s_p1 = ExitStack()
    ptr = s_p1.enter_context(tc.tile_pool(name="ptr", bufs=2, space="PSUM"))
    pj = s_p1.enter_context(tc.tile_pool(name="pj", bufs=3, space="PSUM"))
    xrow_pool = s_p1.enter_context(tc.tile_pool(name="xrow", bufs=2))
    wblk_pool = s_p1.enter_context(tc.tile_pool(name="wblk", bufs=2))
    wrblk_pool = s_p1.enter_context(tc.tile_pool(name="wrblk", bufs=2))

    # xq -> xqT, Q projection, spill xqT (reloaded in P3)
    s_xq = ExitStack()
    pool_xq = s_xq.enter_context(tc.tile_pool(name="pool_xq", bufs=1))
    xqT = pool_xq.tile([P, NS, L], F32R, name="xqT")
    _transpose_in(nc, ptr, xq, xqT, ident, xrow_pool)
    _proj_T(nc, pj, wblk_pool, wrblk_pool, qw, qb_c, xqT, QT)
    nc.sync.dma_start(xqT_spill.ap(), xqT[:].bitcast(F32))
    s_xq.close()

    # xkv -> xkvT, K and V projections
    s_xkv = ExitStack()
    pool_xkv = s_xkv.enter_context(tc.tile_pool(name="pool_xkv", bufs=1))
    xkvT = pool_xkv.tile([P, NS, L], F32R, name="xkvT")
    vb_b = pool_xkv.tile([P, D], F32, name="vb_b")
    nc.sync.dma_start(vb_b[:], vb.ap()[None, :].to_broadcast([P, D]))

    _transpose_in(nc, ptr, xkv, xkvT, ident, xrow_pool)
    _proj_T(nc, pj, wblk_pool, wrblk_pool, kw, kb_c, xkvT, KT)

    # V natural: Vn[t, j] (bf16) = xkvT-blocks (stationary) @ vw (moving)
    s_vw = ExitStack()
    vchunk_pool = s_vw.enter_context(tc.tile_pool(name="vchunk", bufs=1))
    vrchunk_pool = s_vw.enter_context(tc.tile_pool(name="vrchunk", bufs=2))
    vwv = vw.ap().rearrange("(cs p) j -> p cs j", p=P)
    for half in range(4):  # 256-wide moving chunks
        vc = vchunk_pool.tile([P, NS, 256], F32, name="vc", tag="vc")
        nc.sync.dma_start(vc[:], vwv[:, :, half * 256:(half + 1) * 256])
        vr = vrchunk_pool.tile([P, NS, 256], F32R, name="vr", tag="vr")
        nc.vector.tensor_copy(vr[:], vc[:])
        for ts in range(NS):
            ps = pj.tile([P, 256], F32, name="pjv", tag="pjv", space="PSUM")
            for cs in range(NS):
                nc.tensor.matmul(ps[:], xkvT[:, cs, ts * P:(ts + 1) * P],
                                 vr[:, cs, :],
                                 start=(cs == 0), stop=(cs == NS - 1))
            nc.vector.tensor_tensor(
                Vn[:, ts, half * 256:(half + 1) * 256], ps[:],
                vb_b[:, half * 256:(half + 1) * 256], ALU.add)
    s_vw.close()
    s_xkv.close()
    s_p1.close()

    # ================= P2: attention (single merged phase) ==============
    # Per head: S[q,k] matmuls -> exp (bf16 E + rowsum) -> DVE attn-FMA
    # -> DMA-transpose E into ET[k,q] fragments -> aw@V matmuls (bf16).
    # Softmax normalization (1/rowsum) of attn is fused into the FMA; the
    # ctx normalization is deferred to one batched pass at phase end.
    s_A = ExitStack()
    psA = s_A.enter_context(tc.tile_pool(name="psA", bufs=2, space="PSUM"))
    psC = s_A.enter_context(tc.tile_pool(name="psC", bufs=1, space="PSUM"))
    pool_attn = s_A.enter_context(tc.tile_pool(name="pool_attn", bufs=1))
    attn_sb = pool_attn.tile([P, NS, L], F32, name="attn_sb")
    epool = s_A.enter_context(tc.tile_pool(name="epool", bufs=3))
    etpool = s_A.enter_context(tc.tile_pool(name="etpool", bufs=2))
    rsm_pool = s_A.enter_context(tc.tile_pool(name="rsm", bufs=4))

    for h in range(H):
        hs = h // 2
        sub = h % 2
        hp = sub * HD
        qsl = QT[hp:hp + HD, hs, :]
        ksl = KT[hp:hp + HD, hs, :]
        # ET_h[p, qs, ks, q'] = E_h[q', ks*128+p] for q = qs*128+q'
        ET = etpool.tile([P, NS, NS, P], BF16, name="ET", tag="ET")
        ctxp = psC.tile([P, NCH, CH], F32, name=f"ctxp{sub}",
                        tag=f"ctxp{sub}", space="PSUM")
        for qs in range(NS):
            rs_col = rsm_pool.tile([P, 1], F32, name="rs_col", tag="rs_col")
            E = epool.tile([P, L], BF16, name="E", tag="E")
            sA = psA.tile([P, NCH, CH], F32, name="sA", tag="sA",
                          space="PSUM")
            for ch in range(NCH):
                nc.tensor.matmul(sA[:, ch, :], qsl[:, qs * P:(qs + 1) * P],
                                 ksl[:, ch * CH:(ch + 1) * CH],
                                 start=True, stop=True)
            nc.scalar.activation(E[:], sA[:], AF.Exp, scale=cscale[:],
                                 accum_out=rs_col[:])
            nc.vector.reciprocal(r_all[:, h, qs:qs + 1], rs_col[:])
            r16 = rsm_pool.tile([P, 1], F32, name="r16", tag="r16")
            nc.vector.tensor_scalar_mul(r16[:], r_all[:, h, qs:qs + 1],
                                        1.0 / H)
            if h == 0:
                nc.vector.tensor_scalar(attn_sb[:, qs, :], E[:],
                                        r16[:], None, op0=ALU.mult)
            else:
                nc.vector.scalar_tensor_tensor(
                    attn_sb[:, qs, :], E[:], r16[:], attn_sb[:, qs, :],
                    op0=ALU.mult, op1=ALU.add)
            nc.sync.dma_start_transpose(ET[:, qs, :, :], E[:])
        # ctx accumulation: lhsT = V columns of this head (stationary),
        # rhs = ET k-rows (strided across qs fragments)
        for ks in range(NS):
            for ch in range(NCH):
                nc.tensor.matmul(
                    ctxp[hp:hp + HD, ch, :],
                    Vn[:, ks, h * HD:(h + 1) * HD],
                    ET[:, 4 * ch:4 * ch + 4, ks, :],
                    start=(ks == 0), stop=(ks == NS - 1))
        for ch in range(NCH):
            nc.vector.tensor_copy(
                ctxT[hp:hp + HD, hs, ch * CH:(ch + 1) * CH],
                ctxp[hp:hp + HD, ch, :])

    nc.sync.dma_start(attn.ap().rearrange("(qs p) k -> p qs k", p=P),
                      attn_sb[:])
    s_A.close()
    s_qkv.close()

    # ---- batched ctx normalization: ctxT[d, q] *= 1/rowsum[h(d), q] ----
    s_rn = ExitStack()
    psR = s_rn.enter_context(tc.tile_pool(name="psR", bufs=1, space="PSUM"))
    rbpool = s_rn.enter_context(tc.tile_pool(name="rb", bufs=2))
    prt = psR.tile([H * NS, P], F32, name="prt", space="PSUM")
    nc.tensor.transpose(prt[:], r_all[:].rearrange("p a b -> p (a b)"),
                        ident[:])
    rT_all = rbpool.tile([H * NS, P], F32, name="rT_all", tag="rT_all")
    nc.vector.tensor_copy(rT_all[:], prt[:])
    for h in range(H):
        hs = h // 2
        hp = (h % 2) * HD
        r_row = rbpool.tile([1, NS, P], F32, name="r_row", tag="r_row")
        nc.sync.dma_start(r_row[:], rT_all[h * NS:(h + 1) * NS, None, :])
        r_b = rbpool.tile([P, L], F32, name="r_b", tag="r_b")
        nc.gpsimd.partition_broadcast(
            r_b[:], r_row[:].rearrange("a b c -> a (b c)"))
        for ch in range(NCH):
            nc.vector.tensor_tensor(
                ctxT[hp:hp + HD, hs, ch * CH:(ch + 1) * CH],
                ctxT[hp:hp + HD, hs, ch * CH:(ch + 1) * CH].bitcast(F32),
                r_b[hp:hp + HD, ch * CH:(ch + 1) * CH], ALU.mult)
    s_rn.close()

    # ================= P3: out-proj + LN + fusion =================
    # zT[j, q] = ctx @ ow + ob + xq; after LN applied in place, zT == updT
    s_z = ExitStack()
    pool_z = s_z.enter_context(tc.tile_pool(name="pool_z", bufs=1))
    zT = pool_z.tile([P, NS, L], F32R, name="zT")

    s_xq2 = ExitStack()
    pool_xq2 = s_xq2.enter_context(tc.tile_pool(name="pool_xq2", bufs=1))
    xq2 = pool_xq2.tile([P, NS, L], F32R, name="xq2")
    s_rot = ExitStack()
    xq2_rot = s_rot.enter_context(tc.tile_pool(name="xq2rot", bufs=2))
    for ds in range(NS):
        xr_ = xq2_rot.tile([P, L], F32, name="xq2r", tag="xq2r")
        nc.sync.dma_start(xr_[:], xqT_spill.ap()[:, ds, :])
        nc.vector.tensor_copy(xq2[:, ds, :], xr_[:])
    s_rot.close()

    s_zmm = ExitStack()
    zp_ps = s_zmm.enter_context(tc.tile_pool(name="zp", bufs=2, space="PSUM"))
    stat_ps = s_zmm.enter_context(tc.tile_pool(name="stat", bufs=1,
                                               space="PSUM"))
    s_ow = ExitStack()
    wblk_ow = s_ow.enter_context(tc.tile_pool(name="wblk_ow", bufs=2))
    wrblk_ow = s_ow.enter_context(tc.tile_pool(name="wrblk_ow", bufs=2))
    zsq_pool = s_ow.enter_context(tc.tile_pool(name="zsq", bufs=2))

    owv = ow.ap().rearrange("(cs p) j -> p cs j", p=P)
    s1 = stat_ps.tile([2, NCH, CH], F32, name="s1", tag="s1", space="PSUM")
    s2 = stat_ps.tile([2, NCH, CH], F32, name="s2", tag="s2", space="PSUM")
    for js in range(NS):
        wb = wblk_ow.tile([P, NS, P], F32, name="wb2", tag="wb2")
        nc.sync.dma_start(wb[:], owv[:, :, js * P:(js + 1) * P])
        wr = wrblk_ow.tile([P, NS, P], F32R, name="wr2", tag="wr2")
        nc.vector.tensor_copy(wr[:], wb[:])
        for ch in range(NCH):
            ps = zp_ps.tile([P, CH], F32, name="zpp", tag="zpp", space="PSUM")
            for cs in range(NS):
                nc.tensor.matmul(ps[:], wr[:, cs, :],
                                 ctxT[:, cs, ch * CH:(ch + 1) * CH],
                                 start=(cs == 0), stop=(cs == NS - 1))
            nc.vector.scalar_tensor_tensor(
                zT[:, js, ch * CH:(ch + 1) * CH], ps[:], ob_c[:, js:js + 1],
                xq2[:, js, ch * CH:(ch + 1) * CH].bitcast(F32),
                op0=ALU.add, op1=ALU.add)
        # LN sums over feature dim (partitions) via ones-matmul, accum js
        zsq = zsq_pool.tile([P, L], F32R, name="zsq", tag="zsq")
        nc.vector.tensor_tensor(zsq[:],
                                zT[:, js, :].bitcast(F32),
                                zT[:, js, :].bitcast(F32), ALU.mult)
        for ch in range(NCH):
            nc.tensor.matmul(s1[:, ch, :], onesr[:],
                             zT[:, js, ch * CH:(ch + 1) * CH],
                             start=(js == 0), stop=(js == NS - 1))
            nc.tensor.matmul(s2[:, ch, :], onesr[:],
                             zsq[:, ch * CH:(ch + 1) * CH],
                             start=(js == 0), stop=(js == NS - 1))
    s_ow.close()

    # ---- LN stats rows + partition broadcast ----
    s_stat = ExitStack()
    statrows = s_stat.enter_context(tc.tile_pool(name="statrows", bufs=1))
    mean_r = statrows.tile([1, L], F32, name="mean_r")
    rstd_r = statrows.tile([1, L], F32, name="rstd_r")
    for ch in range(NCH):
        sl = slice(ch * CH, (ch + 1) * CH)
        nc.vector.tensor_scalar_mul(mean_r[:, sl], s1[0:1, ch, :], 1.0 / D)
        nc.vector.tensor_scalar_mul(rstd_r[:, sl], s2[0:1, ch, :], 1.0 / D)
        m2 = statrows.tile([1, CH], F32, name="m2", tag="m2")
        nc.vector.tensor_mul(m2[:], mean_r[:, sl], mean_r[:, sl])
        nc.vector.tensor_sub(rstd_r[:, sl], rstd_r[:, sl], m2[:])
        nc.scalar.activation(rstd_r[:, sl], rstd_r[:, sl], AF.Sqrt,
                             bias=eps_t[:])
        nc.vector.reciprocal(rstd_r[:, sl], rstd_r[:, sl])
    mean_b = statrows.tile([P, L], F32, name="mean_b")
    nc.gpsimd.partition_broadcast(mean_b[:], mean_r[:])
    rstd_b = statrows.tile([P, L], F32, name="rstd_b")
    nc.gpsimd.partition_broadcast(rstd_b[:], rstd_r[:])

    # ---- LN apply, in place over zT (zT becomes updT) ----
    s_lnt = ExitStack()
    ln_tmp_pool = s_lnt.enter_context(tc.tile_pool(name="lntmp", bufs=2))
    for ds in range(NS):
        t1 = ln_tmp_pool.tile([P, L], F32, name="t1", tag="t1")
        nc.vector.tensor_sub(t1[:], zT[:, ds, :].bitcast(F32), mean_b[:])
        nc.vector.scalar_tensor_tensor(t1[:], t1[:], g_c[:, ds:ds + 1],
                                       rstd_b[:], op0=ALU.mult, op1=ALU.mult)
        nc.vector.tensor_scalar(zT[:, ds, :], t1[:],
                                beta_c[:, ds:ds + 1], None, op0=ALU.add)
    s_lnt.close()
    s_stat.close()
    s_zmm.close()

    # ---- fusion mm1: g1T[j, t] = gelu(w1.T @ [updT; xqT] + b1) ----
    s_h1 = ExitStack()
    pool_h1 = s_h1.enter_context(tc.tile_pool(name="pool_h1", bufs=1))
    g1T = pool_h1.tile([P, NS, L], F32R, name="g1T")
    s_w1 = ExitStack()
    h1_ps = s_w1.enter_context(tc.tile_pool(name="h1p", bufs=3, space="PSUM"))
    wblk_w1 = s_w1.enter_context(tc.tile_pool(name="wblk_w1", bufs=2))
    wrblk_w1 = s_w1.enter_context(tc.tile_pool(name="wrblk_w1", bufs=2))
    w1v = w1.ap().rearrange("(cs p) j -> p cs j", p=P)
    for js in range(NS):
        wb = wblk_w1.tile([P, 2 * NS, P], F32, name="wb1", tag="wb1")
        nc.sync.dma_start(wb[:], w1v[:, :, js * P:(js + 1) * P])
        wr = wrblk_w1.tile([P, 2 * NS, P], F32R, name="wr1", tag="wr1")
        nc.vector.tensor_copy(wr[:], wb[:])
        for ch in range(NCH):
            ps = h1_ps.tile([P, CH], F32, name="h1pp", tag="h1pp",
                            space="PSUM")
            for cs in range(2 * NS):
                rhs = (zT[:, cs, ch * CH:(ch + 1) * CH] if cs < NS
                       else xq2[:, cs - NS, ch * CH:(ch + 1) * CH])
                nc.tensor.matmul(ps[:], wr[:, cs, :], rhs,
                                 start=(cs == 0), stop=(cs == 2 * NS - 1))
            nc.scalar.activation(g1T[:, js, ch * CH:(ch + 1) * CH], ps[:],
                                 AF.Gelu, bias=b1_c[:, js:js + 1])
    s_w1.close()

    # ---- fusion mm2: fused[t, j2] = g1T-blocks @ w2 + b2 ----
    s_w2 = ExitStack()
    h2_ps = s_w2.enter_context(tc.tile_pool(name="h2p", bufs=3, space="PSUM"))
    out_pool = s_w2.enter_context(tc.tile_pool(name="outp", bufs=3))
    wblk_w2 = s_w2.enter_context(tc.tile_pool(name="wblk_w2", bufs=1))
    wrblk_w2 = s_w2.enter_context(tc.tile_pool(name="wrblk_w2", bufs=2))
    b2pool = s_w2.enter_context(tc.tile_pool(name="b2pool", bufs=1))
    b2_b = b2pool.tile([P, D], F32, name="b2_b")
    nc.sync.dma_start(b2_b[:], b2.ap()[None, :].to_broadcast([P, D]))

    w2v = w2.ap().rearrange("(js p) j2 -> p js j2", p=P)
    fv = _row_tiles(fused.ap())
    for quad in range(4):  # 256-wide output chunks
        w2c = wblk_w2.tile([P, NS, 256], F32, name="w2c", tag="w2c")
        nc.sync.dma_start(w2c[:], w2v[:, :, quad * 256:(quad + 1) * 256])
        w2r = wrblk_w2.tile([P, NS, 256], F32R, name="w2r", tag="w2r")
        nc.vector.tensor_copy(w2r[:], w2c[:])
        for ts in range(NS):
            ps = h2_ps.tile([P, 256], F32, name="h2pp", tag="h2pp",
                            space="PSUM")
            for js2 in range(NS):
                nc.tensor.matmul(ps[:], g1T[:, js2, ts * P:(ts + 1) * P],
                                 w2r[:, js2, :],
                                 start=(js2 == 0), stop=(js2 == NS - 1))
            orow = out_pool.tile([P, 256], F32, name="orow", tag="orow")
            nc.vector.tensor_tensor(
                orow[:], ps[:], b2_b[:, quad * 256:(quad + 1) * 256],
                ALU.add)
            nc.sync.dma_start(fv[:, ts, quad * 256:(quad + 1) * 256],
                              orow[:])
    s_w2.close()
    s_h1.close()
    s_xq2.close()
    s_z.close()
    s_ctx.close()
    s_const.close()


_NC_CACHE = {}


def _get_program():
    if "nc" not in _NC_CACHE:
        _NC_CACHE["nc"] = build_program()
    return _NC_CACHE["nc"]


def _make_in_maps(inp):
    in_maps = []
    for d in ("s2s", "t2s"):
        qk = "seq_features" if d == "s2s" else "struct_features"
        kk = "struct_features" if d == "s2s" else "seq_features"
        for b in range(4):
            in_maps.append({
                "xq": inp[qk][b], "xkv": inp[kk][b],
                "qw": inp[f"{d}_qw"], "kw": inp[f"{d}_kw"],
                "vw": inp[f"{d}_vw"], "ow": inp[f"{d}_ow"],
                "qb": inp[f"{d}_qb"], "kb": inp[f"{d}_kb"],
                "vb": inp[f"{d}_vb"], "ob": inp[f"{d}_ob"],
                "gg": inp[f"{d}_g"], "gbeta": inp[f"{d}_beta"],
                "temp": inp[f"{d}_temp"],
                "w1": inp["fus_w1"], "b1": inp["fus_b1"],
                "w2": inp["fus_w2"], "b2": inp["fus_b2"],
            })
    return in_maps


def kernel(**inputs):
    inp = {k: np.ascontiguousarray(np.asarray(v), dtype=None)
           for k, v in inputs.items()}
    nc = _get_program()
    in_maps = _make_in_maps(inp)
    res = run_bass_kernel_spmd(nc, in_maps, list(range(8))).results

    seq_fused = np.stack([res[b]["fused"] for b in range(4)])
    struct_fused = np.stack([res[4 + b]["fused"] for b in range(4)])
    s2s_attn = np.stack([res[b]["attn"] for b in range(4)])
    t2s_attn = np.stack([res[4 + b]["attn"] for b in range(4)])
    return seq_fused, struct_fused, s2s_attn, t2s_attn
